# revision 10
# baseline (speedup 1.0000x reference)
"""Trainium2 Bass kernel for CNN+Mamba classifier.

Contract: kernel(**inputs) takes FULL unsharded inputs (numpy), returns FULL
(8, 10) float32 output. Internally shards data-parallel over batch across 8
NeuronCores (1 example per core), with all parameters replicated.

Self-contained: hardcodes all shapes; no sibling imports.
"""

import os
from contextlib import ExitStack

import numpy as np
import ml_dtypes

import concourse.bass as bass
import concourse.bacc as bacc
import concourse.tile as tile
from concourse import mybir
from concourse.bass_utils import run_bass_kernel_spmd

FP = mybir.dt.float32
FR = mybir.dt.float32r
BF = mybir.dt.bfloat16
I32 = mybir.dt.int32

VOCAB, EMB, NCLS, SEQ = 50000, 256, 10, 2048
DM, DI, DS, DCONV, DTR = 128, 256, 16, 4, 8
L = SEQ // 2  # 1024 after maxpool
NTILE = DI // 8  # 32 scan tiles, each 8 channels x 16 states


def _rep_ap(t_ap, row0, nrows, rep, outer_rows=True):
    """AP reading `nrows` partition rows starting at row0 of a 2D SBUF tile,
    each repeated `rep` times. outer_rows=True -> dest p = row*rep + k;
    False -> dest p = k*nrows + row."""
    full = t_ap[:]
    pstep = full.ap[0][0]
    free = list(full.ap[1:])
    if outer_rows:
        dims = [[pstep, nrows], [0, rep]]
    else:
        dims = [[0, rep], [pstep, nrows]]
    return bass.AP(tensor=full.tensor, offset=full.offset + row0 * pstep,
                   ap=dims + free)


def _strided_pair(t_ap, n):
    """even/odd stride-2 APs over the free dim of a (128, 2n) tile."""
    full = t_ap[:]
    pstep = full.ap[0][0]
    ev = bass.AP(tensor=full.tensor, offset=full.offset, ap=[[pstep, 128], [2, n]])
    od = bass.AP(tensor=full.tensor, offset=full.offset + 1, ap=[[pstep, 128], [2, n]])
    return ev, od


def build_program():
    nc = bacc.Bacc("TRN2", target_bir_lowering=False, debug=False, num_devices=8)

    # ---- DRAM inputs (per-core) ----
    d_ids = nc.dram_tensor("ids", [SEQ], I32, kind="ExternalInput")
    d_emb = nc.dram_tensor("emb", [VOCAB, EMB], FP, kind="ExternalInput")
    d_c1w = nc.dram_tensor("c1w", [128, 5 * 2 * 128], FR, kind="ExternalInput")
    d_xcw = nc.dram_tensor("xcw", [128, 4 * 2 * 128], FR, kind="ExternalInput")
    d_zw = nc.dram_tensor("zw", [128, 2 * 128], FR, kind="ExternalInput")
    d_xpw = nc.dram_tensor("xpw", [128, 2 * 40], FR, kind="ExternalInput")
    d_dtw = nc.dram_tensor("dtw", [8, 2 * 128], FR, kind="ExternalInput")
    d_wa = nc.dram_tensor("wa", [128, NTILE * 128], FR, kind="ExternalInput")
    d_wr = nc.dram_tensor("wr", [128, 4 * 32], BF, kind="ExternalInput")
    d_opw = nc.dram_tensor("opw", [128, 2 * 128], FR, kind="ExternalInput")
    d_fcw = nc.dram_tensor("fcw", [128, NCLS], FP, kind="ExternalInput")
    d_ident = nc.dram_tensor("ident", [128, 128], FP, kind="ExternalInput")
    d_c1b = nc.dram_tensor("c1b", [128, 1], FP, kind="ExternalInput")
    d_cdb = nc.dram_tensor("cdb", [128, 2], FP, kind="ExternalInput")
    d_dtb = nc.dram_tensor("dtb", [128, 2], FP, kind="ExternalInput")
    d_dvec = nc.dram_tensor("dvec", [128, 2], FP, kind="ExternalInput")
    d_fcb = nc.dram_tensor("fcb", [10, 1], FP, kind="ExternalInput")

    d_out = nc.dram_tensor("out", [NCLS], FP, kind="ExternalOutput")

    Alu = mybir.AluOpType
    Act = mybir.ActivationFunctionType

    with ExitStack() as ctx:
        tc = ctx.enter_context(tile.TileContext(nc))
        W = ctx.enter_context(tc.tile_pool(name="w", bufs=1))

        # ---- load constants ----
        def load(dram, shape, dtype=FP):
            t = W.tile(list(shape), dtype, name=f"w_{dram.name}")
            nc.sync.dma_start(out=t[:], in_=dram[:])
            return t

        c1w = load(d_c1w, (128, 5 * 2 * 128), FR)
        xcw = load(d_xcw, (128, 4 * 2 * 128), FR)
        zw = load(d_zw, (128, 2 * 128), FR)
        xpw = load(d_xpw, (128, 2 * 40), FR)
        dtw = load(d_dtw, (8, 2 * 128), FR)
        wa = load(d_wa, (128, NTILE * 128), FR)
        wr = load(d_wr, (128, 4 * 32), BF)
        opw = load(d_opw, (128, 2 * 128), FR)
        fcw = load(d_fcw, (128, NCLS))
        ident = load(d_ident, (128, 128))
        c1b = load(d_c1b, (128, 1))
        cdb = load(d_cdb, (128, 2))
        dtb = load(d_dtb, (128, 2))
        dvec = load(d_dvec, (128, 2))
        fcb = load(d_fcb, (10, 1))

        # ids -> (128, 16): partition p holds ids[c*128+p] at column c
        ids_sb = W.tile([128, 16], I32)
        ids_src = bass.AP(tensor=d_ids[:].tensor, offset=0, ap=[[1, 128], [128, 16]])
        nc.sync.dma_start(out=ids_sb[:], in_=ids_src)

        # ---- persistent intermediates ----
        x_emb = [W.tile([128, SEQ + 4], FR, name=f"x_emb{_}") for _ in range(2)]  # pad 2 each side
        for h in range(2):
            nc.vector.memset(x_emb[h][:, 0:2].bitcast(FP), 0.0)
            nc.vector.memset(x_emb[h][:, SEQ + 2:SEQ + 4].bitcast(FP), 0.0)
        x_pool = W.tile([128, L + 3], FR)  # pad 3 left (causal dconv)
        nc.vector.memset(x_pool[:, 0:3].bitcast(FP), 0.0)
        relu_sb = W.tile([128, SEQ], FP)
        xs_sb = [W.tile([128, L], FR, name=f"xs_sb{_}") for _ in range(2)]
        sz_sb = [W.tile([128, L], FP, name=f"sz_sb{_}") for _ in range(2)]
        dt_sb = [W.tile([128, L], FR, name=f"dt_sb{_}") for _ in range(2)]
        u_sb = [W.tile([128, L], BF, name=f"u_sb{_}") for _ in range(2)]
        xdbl_sb = W.tile([40, L], FR)
        b_rep = W.tile([128, L], BF)
        c_rep = W.tile([128, L], BF)

        # ================= PHASE 1: embedding gather + transpose ============
        with tc.tile_pool(name="g", bufs=3) as gp, \
             tc.tile_pool(name="gt", bufs=4, space="PSUM") as gtp:
            for c in range(16):
                xg = gp.tile([128, EMB], FP)
                nc.gpsimd.indirect_dma_start(
                    out=xg[:], out_offset=None, in_=d_emb[:],
                    in_offset=bass.IndirectOffsetOnAxis(ap=ids_sb[:, c:c + 1], axis=0))
                for h in range(2):
                    pt = gtp.tile([128, 128], FP)
                    nc.tensor.transpose(out=pt[:], in_=xg[:, 128 * h:128 * (h + 1)],
                                        identity=ident[:])
                    nc.scalar.copy(out=x_emb[h][:, 2 + 128 * c:2 + 128 * (c + 1)],
                                   in_=pt[:])

        # ================= PHASE 2: conv1 + relu + maxpool ==================
        with tc.tile_pool(name="cp", bufs=1, space="PSUM") as cp:
            cps = cp.tile([128, SEQ], FP)  # 4 banks
            for nch in range(4):
                o = 512 * nch
                for k in range(5):
                    for kh in range(2):
                        nc.tensor.matmul(
                            out=cps[:, o:o + 512],
                            lhsT=c1w[:, (k * 2 + kh) * 128:(k * 2 + kh + 1) * 128],
                            rhs=x_emb[kh][:, o + k:o + k + 512],
                            start=(k == 0 and kh == 0), stop=(k == 4 and kh == 1))
            nc.scalar.activation(out=relu_sb[:], in_=cps[:], func=Act.Relu,
                                 bias=c1b[:, 0:1], scale=1.0)
        ev, od = _strided_pair(relu_sb, L)
        nc.vector.tensor_max(out=x_pool[:, 3:3 + L], in0=ev, in1=od)

        # ============ PHASE 3: in_proj (+ folded depthwise conv) + silu =====
        with tc.tile_pool(name="ip", bufs=1, space="PSUM") as ip:
            xcp = [ip.tile([128, L], FP, name=f"xcp{_}") for _ in range(2)]
            zp = [ip.tile([128, L], FP, name=f"zp{_}") for _ in range(2)]
            for h in range(2):
                for nch in range(2):
                    o = 512 * nch
                    for k in range(4):
                        nc.tensor.matmul(
                            out=xcp[h][:, o:o + 512],
                            lhsT=xcw[:, (k * 2 + h) * 128:(k * 2 + h + 1) * 128],
                            rhs=x_pool[:, o + k:o + k + 512],
                            start=(k == 0), stop=(k == 3))
                    nc.tensor.matmul(
                        out=zp[h][:, o:o + 512],
                        lhsT=zw[:, h * 128:(h + 1) * 128],
                        rhs=x_pool[:, 3 + o:3 + o + 512],
                        start=True, stop=True)
            for h in range(2):
                nc.scalar.activation(out=xs_sb[h][:], in_=xcp[h][:], func=Act.Silu,
                                     bias=cdb[:, h:h + 1], scale=1.0)
                nc.scalar.activation(out=sz_sb[h][:], in_=zp[h][:], func=Act.Silu,
                                     scale=1.0)

        # ================= PHASE 4: x_proj -> (dt_in, B, C) =================
        with tc.tile_pool(name="xp", bufs=1, space="PSUM") as xp:
            xdp = xp.tile([40, L], FP)
            for kh in range(2):
                for nch in range(2):
                    o = 512 * nch
                    nc.tensor.matmul(
                        out=xdp[:, o:o + 512],
                        lhsT=xpw[:, kh * 40:(kh + 1) * 40],
                        rhs=xs_sb[kh][:, o:o + 512],
                        start=(kh == 0), stop=(kh == 1))
            nc.vector.tensor_copy(out=xdbl_sb[:], in_=xdp[0:40, :])

        # ================= PHASE 5: dt (softplus) + u = dt*xs ===============
        with tc.tile_pool(name="dp", bufs=1, space="PSUM") as dp:
            dtp = [dp.tile([128, L], FP, name=f"dtp{_}") for _ in range(2)]
            for h in range(2):
                for nch in range(2):
                    o = 512 * nch
                    nc.tensor.matmul(
                        out=dtp[h][:, o:o + 512],
                        lhsT=dtw[0:8, h * 128:(h + 1) * 128],
                        rhs=xdbl_sb[0:8, o:o + 512],
                        start=True, stop=True)
            for h in range(2):
                # softplus(x+b) = ln(1 + exp(x+b)); x ~ -4 so no overflow
                nc.scalar.activation(out=dt_sb[h][:], in_=dtp[h][:], func=Act.Exp,
                                     bias=dtb[:, h:h + 1], scale=1.0)
                nc.scalar.activation(out=dt_sb[h][:], in_=dt_sb[h][:].bitcast(FP), func=Act.Ln,
                                     bias=1.0, scale=1.0)
        for h in range(2):
            nc.vector.tensor_mul(out=u_sb[h][:], in0=dt_sb[h][:].bitcast(FP), in1=xs_sb[h][:].bitcast(FP))

        # ================= PHASE 6: replicate B, C across channel groups ====
        # dest p = dl*16 + n  <-  bc row n (B: rows 0:16, C: rows 16:32)
        for dl in range(8):
            nc.gpsimd.dma_start(out=b_rep[dl * 16:(dl + 1) * 16, :],
                                in_=xdbl_sb[8:24, :].bitcast(FP))
            nc.gpsimd.dma_start(out=c_rep[dl * 16:(dl + 1) * 16, :],
                                in_=xdbl_sb[24:40, :].bitcast(FP))

        # ================= PHASE 7: selective scan ==========================
        with tc.tile_pool(name="yp", bufs=1, space="PSUM") as ypp, \
             tc.tile_pool(name="sg", bufs=2, space="PSUM") as sgp, \
             tc.tile_pool(name="sc", bufs=2) as scp:
            yp = [ypp.tile([128, L], FP, name=f"yp{_}") for _ in range(2)]  # 4 banks
            for i in range(NTILE):
                hh = i // 16
                lc = 8 * (i % 16)          # local channel base within half
                g = lc // 32               # 32-partition output group
                o = lc % 32                # offset inside group (0/8/16/24)
                v = o // 8                 # wr variant
                sneg = sgp.tile([128, L], FP)
                for nch in range(2):
                    off = 512 * nch
                    nc.tensor.matmul(
                        out=sneg[:, off:off + 512],
                        lhsT=wa[:, i * 128:(i + 1) * 128],
                        rhs=dt_sb[hh][:, off:off + 512],
                        start=True, stop=True)
                dA = scp.tile([128, L], FP, tag="dA")
                nc.scalar.activation(out=dA[:], in_=sneg[:], func=Act.Exp, scale=1.0)
                urep = scp.tile([128, L], BF, tag="urep")
                nc.sync.dma_start(out=urep[:], in_=_rep_ap(u_sb[hh], lc, 8, 16,
                                                           outer_rows=True))
                dBu = scp.tile([128, L], BF, tag="dBu")
                nc.vector.tensor_mul(out=dBu[:], in0=urep[:], in1=b_rep[:])
                ht = scp.tile([128, L], BF, tag="ht")
                nc.vector.tensor_tensor_scan(out=ht[:], data0=dA[:], data1=dBu[:],
                                             initial=0.0, op0=Alu.mult, op1=Alu.add)
                hC = scp.tile([128, L], BF, tag="hC")
                nc.gpsimd.tensor_mul(out=hC[:], in0=ht[:], in1=c_rep[:])
                for nch in range(2):
                    off = 512 * nch
                    nc.tensor.matmul(
                        out=yp[hh][32 * g:32 * (g + 1), off:off + 512],
                        lhsT=wr[:, v * 32:(v + 1) * 32],
                        rhs=hC[:, off:off + 512],
                        start=(o == 0), stop=(o == 24),
                        tile_position=(0, 32 * g))

            # ============= PHASE 8: gate, out_proj, mean, fc ================
            y2 = [W.tile([128, L], FR, name=f"y2{_}") for _ in range(2)]
            for h in range(2):
                y1 = scp.tile([128, L], FP, tag="y1")
                nc.vector.scalar_tensor_tensor(out=y1[:], in0=xs_sb[h][:].bitcast(FP),
                                               scalar=dvec[:, h:h + 1], in1=yp[h][:],
                                               op0=Alu.mult, op1=Alu.add)
                nc.vector.tensor_mul(out=y2[h][:], in0=y1[:], in1=sz_sb[h][:])

        with tc.tile_pool(name="op", bufs=1, space="PSUM") as opp:
            yop = opp.tile([128, L], FP)
            for h in range(2):
                for nch in range(2):
                    o = 512 * nch
                    nc.tensor.matmul(
                        out=yop[:, o:o + 512],
                        lhsT=opw[:, h * 128:(h + 1) * 128],
                        rhs=y2[h][:, o:o + 512],
                        start=(h == 0), stop=(h == 1))
            ymean = W.tile([128, 1], FP)
            nc.vector.tensor_reduce(out=ymean[:], in_=yop[:],
                                    axis=mybir.AxisListType.X, op=Alu.add)
            fcp = opp.tile([10, 1], FP)
            nc.tensor.matmul(out=fcp[:], lhsT=fcw[:, 0:NCLS], rhs=ymean[:],
                             start=True, stop=True)
            out_sb = W.tile([10, 1], FP)
            nc.vector.tensor_scalar_add(out=out_sb[:], in0=fcp[:],
                                        scalar1=fcb[0:10, 0:1])
        out_dst = bass.AP(tensor=d_out[:].tensor, offset=0, ap=[[1, NCLS]])
        out_src = bass.AP(tensor=out_sb[:].tensor, offset=out_sb[:].offset,
                          ap=[[out_sb[:].ap[0][0], NCLS]])
        nc.sync.dma_start(out=out_dst, in_=out_src)

    nc.compile()
    return nc


def prep_consts(inputs):
    """Host-side weight transforms (parameters only, no data-dependent work)."""
    f32 = np.float32
    emb = np.ascontiguousarray(inputs["emb"], dtype=f32)
    conv1_w = np.asarray(inputs["conv1_w"], f32)      # (128, 256, 5)
    conv1_b = np.asarray(inputs["conv1_b"], f32)
    in_proj_w = np.asarray(inputs["in_proj_w"], f32)  # (512, 128)
    convd_w = np.asarray(inputs["convd_w"], f32)      # (256, 1, 4)
    convd_b = np.asarray(inputs["convd_b"], f32)
    x_proj_w = np.asarray(inputs["x_proj_w"], f32)    # (40, 256)
    dt_proj_w = np.asarray(inputs["dt_proj_w"], f32)  # (256, 8)
    dt_proj_b = np.asarray(inputs["dt_proj_b"], f32)
    A_log = np.asarray(inputs["A_log"], f32)          # (256, 16)
    Dv = np.asarray(inputs["D"], f32)
    out_proj_w = np.asarray(inputs["out_proj_w"], f32)  # (128, 256)
    fc_w = np.asarray(inputs["fc_w"], f32)            # (10, 128)
    fc_b = np.asarray(inputs["fc_b"], f32)

    c1w = np.zeros((128, 5, 2, 128), f32)
    for k in range(5):
        for kh in range(2):
            # lhsT[p, m] = conv1_w[m, kh*128+p, k]
            c1w[:, k, kh, :] = conv1_w[:, kh * 128:(kh + 1) * 128, k].T
    c1w = c1w.reshape(128, -1)

    Wx = in_proj_w[:DI]          # (256, 128)
    xcw = np.zeros((128, 4, 2, 128), f32)
    for k in range(4):
        Wxk = convd_w[:, 0, k][:, None] * Wx          # (256, 128)
        for mc in range(2):
            xcw[:, k, mc, :] = Wxk[mc * 128:(mc + 1) * 128, :].T
    xcw = xcw.reshape(128, -1)

    Wz = in_proj_w[DI:]
    zw = np.zeros((128, 2, 128), f32)
    for mc in range(2):
        zw[:, mc, :] = Wz[mc * 128:(mc + 1) * 128, :].T
    zw = zw.reshape(128, -1)

    xpw = np.zeros((128, 2, 40), f32)
    for kh in range(2):
        xpw[:, kh, :] = x_proj_w[:, kh * 128:(kh + 1) * 128].T
    xpw = xpw.reshape(128, -1)

    dtw = np.zeros((8, 2, 128), f32)
    for mc in range(2):
        dtw[:, mc, :] = dt_proj_w[mc * 128:(mc + 1) * 128, :].T
    dtw = dtw.reshape(8, -1)

    A = -np.exp(A_log)           # (256, 16)
    wa = np.zeros((128, NTILE, 128), f32)
    for j in range(NTILE):
        base = 8 * j
        for dl in range(8):
            p = (base % 128) + dl
            for n in range(DS):
                wa[p, j, dl * 16 + n] = A[base + dl, n]
    wa = wa.reshape(128, -1)

    wr = np.zeros((128, 4, 32), f32)
    for v in range(4):
        for p in range(128):
            wr[p, v, 8 * v + p // 16] = 1.0
    wr = wr.reshape(128, -1).astype(ml_dtypes.bfloat16)

    opw = np.zeros((128, 2, 128), f32)
    for kh in range(2):
        opw[:, kh, :] = out_proj_w[:, kh * 128:(kh + 1) * 128].T
    opw = opw.reshape(128, -1)

    fcw = (fc_w / float(L)).T.copy()                  # (128, 10)

    consts = {
        "emb": emb,
        "c1w": c1w, "xcw": xcw, "zw": zw, "xpw": xpw, "dtw": dtw,
        "wa": wa, "wr": wr, "opw": opw, "fcw": fcw,
        "ident": np.eye(128, dtype=f32),
        "c1b": conv1_b.reshape(128, 1).copy(),
        "cdb": convd_b.reshape(2, 128).T.copy(),
        "dtb": dt_proj_b.reshape(2, 128).T.copy(),
        "dvec": Dv.reshape(2, 128).T.copy(),
        "fcb": fc_b.reshape(10, 1).copy(),
    }
    return consts


_CACHE = {}


def kernel(**inputs) -> np.ndarray:
    ids = np.asarray(inputs["ids"])
    assert ids.shape == (8, SEQ), ids.shape
    ids32 = np.ascontiguousarray(ids, dtype=np.int32)

    if "nc" not in _CACHE:
        _CACHE["nc"] = build_program()
    nc = _CACHE["nc"]

    consts = prep_consts(inputs)
    in_maps = []
    for b in range(8):
        m = dict(consts)
        m["ids"] = ids32[b].copy()
        in_maps.append(m)

    trace = os.environ.get("MAMBA_TRACE", "0") == "1"
    res = run_bass_kernel_spmd(nc, in_maps, core_ids=list(range(8)), trace=trace)
    _CACHE["last_results"] = res
    out = np.stack([res.results[b]["out"] for b in range(8)]).astype(np.float32)
    return out


# revision 11
# speedup vs baseline: 1.1166x; 1.1166x over previous
"""Trainium2 Bass kernel for CNN+Mamba classifier.

Contract: kernel(**inputs) takes FULL unsharded inputs (numpy), returns FULL
(8, 10) float32 output. Internally shards data-parallel over batch across 8
NeuronCores (1 example per core), with all parameters replicated.

Self-contained: hardcodes all shapes; no sibling imports.
"""

import os
from contextlib import ExitStack

import numpy as np
import ml_dtypes

import concourse.bass as bass
import concourse.bacc as bacc
import concourse.tile as tile
from concourse import mybir
from concourse.bass_utils import run_bass_kernel_spmd

FP = mybir.dt.float32
FR = mybir.dt.float32r
BF = mybir.dt.bfloat16
I32 = mybir.dt.int32

VOCAB, EMB, NCLS, SEQ = 50000, 256, 10, 2048
DM, DI, DS, DCONV, DTR = 128, 256, 16, 4, 8
L = SEQ // 2  # 1024 after maxpool
NTILE = DI // 8  # 32 scan tiles, each 8 channels x 16 states


def _rep_ap(t_ap, row0, nrows, rep, outer_rows=True):
    """AP reading `nrows` partition rows starting at row0 of a 2D SBUF tile,
    each repeated `rep` times. outer_rows=True -> dest p = row*rep + k;
    False -> dest p = k*nrows + row."""
    full = t_ap[:]
    pstep = full.ap[0][0]
    free = list(full.ap[1:])
    if outer_rows:
        dims = [[pstep, nrows], [0, rep]]
    else:
        dims = [[0, rep], [pstep, nrows]]
    return bass.AP(tensor=full.tensor, offset=full.offset + row0 * pstep,
                   ap=dims + free)


def _strided_pair(t_ap, n):
    """even/odd stride-2 APs over the free dim of a (128, 2n) tile."""
    full = t_ap[:]
    pstep = full.ap[0][0]
    ev = bass.AP(tensor=full.tensor, offset=full.offset, ap=[[pstep, 128], [2, n]])
    od = bass.AP(tensor=full.tensor, offset=full.offset + 1, ap=[[pstep, 128], [2, n]])
    return ev, od


def build_program():
    nc = bacc.Bacc("TRN2", target_bir_lowering=False, debug=False, num_devices=8)

    # ---- DRAM inputs (per-core) ----
    d_ids = nc.dram_tensor("ids", [SEQ], I32, kind="ExternalInput")
    d_emb = nc.dram_tensor("emb", [VOCAB, EMB], FP, kind="ExternalInput")
    d_c1w = nc.dram_tensor("c1w", [128, 5 * 2 * 128], FR, kind="ExternalInput")
    d_xcw = nc.dram_tensor("xcw", [128, 4 * 2 * 128], FR, kind="ExternalInput")
    d_zw = nc.dram_tensor("zw", [128, 2 * 128], FR, kind="ExternalInput")
    d_xpw = nc.dram_tensor("xpw", [128, 2 * 40], FR, kind="ExternalInput")
    d_dtw = nc.dram_tensor("dtw", [8, 2 * 128], FR, kind="ExternalInput")
    d_wsel = nc.dram_tensor("wsel", [128, 16 * 128], FR, kind="ExternalInput")
    d_wselb = nc.dram_tensor("wselb", [128, 16 * 128], BF, kind="ExternalInput")
    d_asc = nc.dram_tensor("asc", [128, NTILE], FP, kind="ExternalInput")
    d_wr = nc.dram_tensor("wr", [128, 4 * 32], BF, kind="ExternalInput")
    d_opw = nc.dram_tensor("opw", [128, 2 * 128], FR, kind="ExternalInput")
    d_fcw = nc.dram_tensor("fcw", [128, NCLS], FP, kind="ExternalInput")
    d_ident = nc.dram_tensor("ident", [128, 128], FP, kind="ExternalInput")
    d_c1b = nc.dram_tensor("c1b", [128, 1], FP, kind="ExternalInput")
    d_cdb = nc.dram_tensor("cdb", [128, 2], FP, kind="ExternalInput")
    d_dtb = nc.dram_tensor("dtb", [128, 2], FP, kind="ExternalInput")
    d_dvec = nc.dram_tensor("dvec", [128, 2], FP, kind="ExternalInput")
    d_fcb = nc.dram_tensor("fcb", [10, 1], FP, kind="ExternalInput")

    d_out = nc.dram_tensor("out", [NCLS], FP, kind="ExternalOutput")

    Alu = mybir.AluOpType
    Act = mybir.ActivationFunctionType

    with ExitStack() as ctx:
        tc = ctx.enter_context(tile.TileContext(nc))
        W = ctx.enter_context(tc.tile_pool(name="w", bufs=1))

        # ---- load constants ----
        def load(dram, shape, dtype=FP):
            t = W.tile(list(shape), dtype, name=f"w_{dram.name}")
            nc.sync.dma_start(out=t[:], in_=dram[:])
            return t

        c1w = load(d_c1w, (128, 5 * 2 * 128), FR)
        xcw = load(d_xcw, (128, 4 * 2 * 128), FR)
        zw = load(d_zw, (128, 2 * 128), FR)
        xpw = load(d_xpw, (128, 2 * 40), FR)
        dtw = load(d_dtw, (8, 2 * 128), FR)
        wsel = load(d_wsel, (128, 16 * 128), FR)
        wselb = load(d_wselb, (128, 16 * 128), BF)
        asc = load(d_asc, (128, NTILE))
        wr = load(d_wr, (128, 4 * 32), BF)
        opw = load(d_opw, (128, 2 * 128), FR)
        fcw = load(d_fcw, (128, NCLS))
        ident = load(d_ident, (128, 128))
        c1b = load(d_c1b, (128, 1))
        cdb = load(d_cdb, (128, 2))
        dtb = load(d_dtb, (128, 2))
        dvec = load(d_dvec, (128, 2))
        fcb = load(d_fcb, (10, 1))

        # ids -> (128, 16): partition p holds ids[c*128+p] at column c
        ids_sb = W.tile([128, 16], I32)
        ids_src = bass.AP(tensor=d_ids[:].tensor, offset=0, ap=[[1, 128], [128, 16]])
        nc.sync.dma_start(out=ids_sb[:], in_=ids_src)

        # ---- persistent intermediates ----
        x_emb = [W.tile([128, SEQ + 4], FR, name=f"x_emb{_}") for _ in range(2)]  # pad 2 each side
        for h in range(2):
            nc.vector.memset(x_emb[h][:, 0:2].bitcast(FP), 0.0)
            nc.vector.memset(x_emb[h][:, SEQ + 2:SEQ + 4].bitcast(FP), 0.0)
        x_pool = W.tile([128, L + 3], FR)  # pad 3 left (causal dconv)
        nc.vector.memset(x_pool[:, 0:3].bitcast(FP), 0.0)
        relu_sb = W.tile([128, SEQ], FP)
        xs_sb = [W.tile([128, L], FR, name=f"xs_sb{_}") for _ in range(2)]
        sz_sb = [W.tile([128, L], FP, name=f"sz_sb{_}") for _ in range(2)]
        dt_sb = [W.tile([128, L], FR, name=f"dt_sb{_}") for _ in range(2)]
        u_sb = [W.tile([128, L], BF, name=f"u_sb{_}") for _ in range(2)]
        xdbl_sb = W.tile([40, L], FR)
        b_rep = W.tile([128, L], BF)
        c_rep = W.tile([128, L], BF)

        # ================= PHASE 1: embedding gather + transpose ============
        with tc.tile_pool(name="g", bufs=3) as gp, \
             tc.tile_pool(name="gt", bufs=4, space="PSUM") as gtp:
            for c in range(16):
                xg = gp.tile([128, EMB], FP)
                nc.gpsimd.indirect_dma_start(
                    out=xg[:], out_offset=None, in_=d_emb[:],
                    in_offset=bass.IndirectOffsetOnAxis(ap=ids_sb[:, c:c + 1], axis=0))
                for h in range(2):
                    pt = gtp.tile([128, 128], FP)
                    nc.tensor.transpose(out=pt[:], in_=xg[:, 128 * h:128 * (h + 1)],
                                        identity=ident[:])
                    nc.scalar.copy(out=x_emb[h][:, 2 + 128 * c:2 + 128 * (c + 1)],
                                   in_=pt[:])

        # ================= PHASE 2: conv1 + relu + maxpool ==================
        with tc.tile_pool(name="cp", bufs=1, space="PSUM") as cp:
            cps = cp.tile([128, SEQ], FP)  # 4 banks
            for nch in range(4):
                o = 512 * nch
                for k in range(5):
                    for kh in range(2):
                        nc.tensor.matmul(
                            out=cps[:, o:o + 512],
                            lhsT=c1w[:, (k * 2 + kh) * 128:(k * 2 + kh + 1) * 128],
                            rhs=x_emb[kh][:, o + k:o + k + 512],
                            start=(k == 0 and kh == 0), stop=(k == 4 and kh == 1))
            nc.scalar.activation(out=relu_sb[:], in_=cps[:], func=Act.Relu,
                                 bias=c1b[:, 0:1], scale=1.0)
        ev, od = _strided_pair(relu_sb, L)
        nc.vector.tensor_max(out=x_pool[:, 3:3 + L], in0=ev, in1=od)

        # ============ PHASE 3: in_proj (+ folded depthwise conv) + silu =====
        with tc.tile_pool(name="ip", bufs=1, space="PSUM") as ip:
            xcp = [ip.tile([128, L], FP, name=f"xcp{_}") for _ in range(2)]
            zp = [ip.tile([128, L], FP, name=f"zp{_}") for _ in range(2)]
            for h in range(2):
                for nch in range(2):
                    o = 512 * nch
                    for k in range(4):
                        nc.tensor.matmul(
                            out=xcp[h][:, o:o + 512],
                            lhsT=xcw[:, (k * 2 + h) * 128:(k * 2 + h + 1) * 128],
                            rhs=x_pool[:, o + k:o + k + 512],
                            start=(k == 0), stop=(k == 3))
                    nc.tensor.matmul(
                        out=zp[h][:, o:o + 512],
                        lhsT=zw[:, h * 128:(h + 1) * 128],
                        rhs=x_pool[:, 3 + o:3 + o + 512],
                        start=True, stop=True)
            for h in range(2):
                nc.scalar.activation(out=xs_sb[h][:], in_=xcp[h][:], func=Act.Silu,
                                     bias=cdb[:, h:h + 1], scale=1.0)
                nc.scalar.activation(out=sz_sb[h][:], in_=zp[h][:], func=Act.Silu,
                                     scale=1.0)

        # ================= PHASE 4: x_proj -> (dt_in, B, C) =================
        with tc.tile_pool(name="xp", bufs=1, space="PSUM") as xp:
            xdp = xp.tile([40, L], FP)
            for kh in range(2):
                for nch in range(2):
                    o = 512 * nch
                    nc.tensor.matmul(
                        out=xdp[:, o:o + 512],
                        lhsT=xpw[:, kh * 40:(kh + 1) * 40],
                        rhs=xs_sb[kh][:, o:o + 512],
                        start=(kh == 0), stop=(kh == 1))
            nc.vector.tensor_copy(out=xdbl_sb[:], in_=xdp[0:40, :])

        # ================= PHASE 5: dt (softplus) + u = dt*xs ===============
        with tc.tile_pool(name="dp", bufs=1, space="PSUM") as dp:
            dtp = [dp.tile([128, L], FP, name=f"dtp{_}") for _ in range(2)]
            for h in range(2):
                for nch in range(2):
                    o = 512 * nch
                    nc.tensor.matmul(
                        out=dtp[h][:, o:o + 512],
                        lhsT=dtw[0:8, h * 128:(h + 1) * 128],
                        rhs=xdbl_sb[0:8, o:o + 512],
                        start=True, stop=True)
            for h in range(2):
                # softplus(x+b) = ln(1 + exp(x+b)); x ~ -4 so no overflow
                nc.scalar.activation(out=dt_sb[h][:], in_=dtp[h][:], func=Act.Exp,
                                     bias=dtb[:, h:h + 1], scale=1.0)
                nc.scalar.activation(out=dt_sb[h][:], in_=dt_sb[h][:].bitcast(FP), func=Act.Ln,
                                     bias=1.0, scale=1.0)
        for h in range(2):
            nc.vector.tensor_mul(out=u_sb[h][:], in0=dt_sb[h][:].bitcast(FP), in1=xs_sb[h][:].bitcast(FP))

        # ================= PHASE 6: replicate B, C across channel groups ====
        # dest p = dl*16 + n  <-  bc row n (B: rows 0:16, C: rows 16:32)
        for dl in range(8):
            nc.gpsimd.dma_start(out=b_rep[dl * 16:(dl + 1) * 16, :],
                                in_=xdbl_sb[8:24, :].bitcast(FP))
            nc.gpsimd.dma_start(out=c_rep[dl * 16:(dl + 1) * 16, :],
                                in_=xdbl_sb[24:40, :].bitcast(FP))

        # ================= PHASE 7: selective scan ==========================
        with tc.tile_pool(name="yp", bufs=1, space="PSUM") as ypp, \
             tc.tile_pool(name="sg", bufs=1, space="PSUM") as sgp, \
             tc.tile_pool(name="sc", bufs=2) as scp:
            yp = [ypp.tile([128, L], FP, name=f"yp{_}") for _ in range(2)]  # 4 banks
            for i in range(NTILE):
                hh = i // 16
                lc = 8 * (i % 16)          # local channel base within half
                g = lc // 32               # 32-partition output group
                o = lc % 32                # offset inside group (0/8/16/24)
                v = o // 8                 # wr variant
                j = i % 16                 # selection variant
                dt_ps = sgp.tile([128, L], FP, tag="dt_ps")
                u_ps = sgp.tile([128, L], FP, tag="u_ps")
                for nch in range(2):
                    off = 512 * nch
                    nc.tensor.matmul(
                        out=dt_ps[:, off:off + 512],
                        lhsT=wsel[:, j * 128:(j + 1) * 128],
                        rhs=dt_sb[hh][:, off:off + 512],
                        start=True, stop=True)
                    nc.tensor.matmul(
                        out=u_ps[:, off:off + 512],
                        lhsT=wselb[:, j * 128:(j + 1) * 128],
                        rhs=u_sb[hh][:, off:off + 512],
                        start=True, stop=True)
                dA = scp.tile([128, L], BF, tag="dA")
                nc.scalar.activation(out=dA[:], in_=dt_ps[:], func=Act.Exp,
                                     scale=asc[:, i:i + 1])
                dBu = scp.tile([128, L], BF, tag="dBu")
                nc.vector.tensor_mul(out=dBu[:], in0=u_ps[:], in1=b_rep[:])
                ht = scp.tile([128, L], BF, tag="ht")
                nc.vector.tensor_tensor_scan(out=ht[:], data0=dA[:], data1=dBu[:],
                                             initial=0.0, op0=Alu.mult, op1=Alu.add)
                hC = scp.tile([128, L], BF, tag="hC")
                if i % 2 == 0:
                    nc.gpsimd.tensor_mul(out=hC[:], in0=ht[:], in1=c_rep[:])
                else:
                    nc.vector.tensor_mul(out=hC[:], in0=ht[:], in1=c_rep[:])
                for nch in range(2):
                    off = 512 * nch
                    nc.tensor.matmul(
                        out=yp[hh][32 * g:32 * (g + 1), off:off + 512],
                        lhsT=wr[:, v * 32:(v + 1) * 32],
                        rhs=hC[:, off:off + 512],
                        start=(o == 0), stop=(o == 24),
                        tile_position=(0, 32 * g))

            # ============= PHASE 8: gate, out_proj, mean, fc ================
            y2 = [W.tile([128, L], FR, name=f"y2{_}") for _ in range(2)]
            for h in range(2):
                y1 = scp.tile([128, L], FP, tag="y1")
                nc.vector.scalar_tensor_tensor(out=y1[:], in0=xs_sb[h][:].bitcast(FP),
                                               scalar=dvec[:, h:h + 1], in1=yp[h][:],
                                               op0=Alu.mult, op1=Alu.add)
                nc.vector.tensor_mul(out=y2[h][:], in0=y1[:], in1=sz_sb[h][:])

        with tc.tile_pool(name="op", bufs=1, space="PSUM") as opp:
            yop = opp.tile([128, L], FP)
            for h in range(2):
                for nch in range(2):
                    o = 512 * nch
                    nc.tensor.matmul(
                        out=yop[:, o:o + 512],
                        lhsT=opw[:, h * 128:(h + 1) * 128],
                        rhs=y2[h][:, o:o + 512],
                        start=(h == 0), stop=(h == 1))
            ymean = W.tile([128, 1], FP)
            nc.vector.tensor_reduce(out=ymean[:], in_=yop[:],
                                    axis=mybir.AxisListType.X, op=Alu.add)
            fcp = opp.tile([10, 1], FP)
            nc.tensor.matmul(out=fcp[:], lhsT=fcw[:, 0:NCLS], rhs=ymean[:],
                             start=True, stop=True)
            out_sb = W.tile([10, 1], FP)
            nc.vector.tensor_scalar_add(out=out_sb[:], in0=fcp[:],
                                        scalar1=fcb[0:10, 0:1])
        out_dst = bass.AP(tensor=d_out[:].tensor, offset=0, ap=[[1, NCLS]])
        out_src = bass.AP(tensor=out_sb[:].tensor, offset=out_sb[:].offset,
                          ap=[[out_sb[:].ap[0][0], NCLS]])
        nc.sync.dma_start(out=out_dst, in_=out_src)

    nc.compile()
    return nc


def prep_consts(inputs):
    """Host-side weight transforms (parameters only, no data-dependent work)."""
    f32 = np.float32
    emb = np.ascontiguousarray(inputs["emb"], dtype=f32)
    conv1_w = np.asarray(inputs["conv1_w"], f32)      # (128, 256, 5)
    conv1_b = np.asarray(inputs["conv1_b"], f32)
    in_proj_w = np.asarray(inputs["in_proj_w"], f32)  # (512, 128)
    convd_w = np.asarray(inputs["convd_w"], f32)      # (256, 1, 4)
    convd_b = np.asarray(inputs["convd_b"], f32)
    x_proj_w = np.asarray(inputs["x_proj_w"], f32)    # (40, 256)
    dt_proj_w = np.asarray(inputs["dt_proj_w"], f32)  # (256, 8)
    dt_proj_b = np.asarray(inputs["dt_proj_b"], f32)
    A_log = np.asarray(inputs["A_log"], f32)          # (256, 16)
    Dv = np.asarray(inputs["D"], f32)
    out_proj_w = np.asarray(inputs["out_proj_w"], f32)  # (128, 256)
    fc_w = np.asarray(inputs["fc_w"], f32)            # (10, 128)
    fc_b = np.asarray(inputs["fc_b"], f32)

    c1w = np.zeros((128, 5, 2, 128), f32)
    for k in range(5):
        for kh in range(2):
            # lhsT[p, m] = conv1_w[m, kh*128+p, k]
            c1w[:, k, kh, :] = conv1_w[:, kh * 128:(kh + 1) * 128, k].T
    c1w = c1w.reshape(128, -1)

    Wx = in_proj_w[:DI]          # (256, 128)
    xcw = np.zeros((128, 4, 2, 128), f32)
    for k in range(4):
        Wxk = convd_w[:, 0, k][:, None] * Wx          # (256, 128)
        for mc in range(2):
            xcw[:, k, mc, :] = Wxk[mc * 128:(mc + 1) * 128, :].T
    xcw = xcw.reshape(128, -1)

    Wz = in_proj_w[DI:]
    zw = np.zeros((128, 2, 128), f32)
    for mc in range(2):
        zw[:, mc, :] = Wz[mc * 128:(mc + 1) * 128, :].T
    zw = zw.reshape(128, -1)

    xpw = np.zeros((128, 2, 40), f32)
    for kh in range(2):
        xpw[:, kh, :] = x_proj_w[:, kh * 128:(kh + 1) * 128].T
    xpw = xpw.reshape(128, -1)

    dtw = np.zeros((8, 2, 128), f32)
    for mc in range(2):
        dtw[:, mc, :] = dt_proj_w[mc * 128:(mc + 1) * 128, :].T
    dtw = dtw.reshape(8, -1)

    A = -np.exp(A_log)           # (256, 16)
    wsel = np.zeros((128, 16, 128), f32)
    for j in range(16):
        lc = 8 * j
        for dl in range(8):
            for n in range(DS):
                wsel[lc + dl, j, dl * 16 + n] = 1.0
    wsel = wsel.reshape(128, -1)
    asc = np.zeros((128, NTILE), f32)
    for i in range(NTILE):
        for p in range(128):
            asc[p, i] = A[8 * i + p // 16, p % 16]

    wr = np.zeros((128, 4, 32), f32)
    for v in range(4):
        for p in range(128):
            wr[p, v, 8 * v + p // 16] = 1.0
    wr = wr.reshape(128, -1).astype(ml_dtypes.bfloat16)

    opw = np.zeros((128, 2, 128), f32)
    for kh in range(2):
        opw[:, kh, :] = out_proj_w[:, kh * 128:(kh + 1) * 128].T
    opw = opw.reshape(128, -1)

    fcw = (fc_w / float(L)).T.copy()                  # (128, 10)

    consts = {
        "emb": emb,
        "c1w": c1w, "xcw": xcw, "zw": zw, "xpw": xpw, "dtw": dtw,
        "wsel": wsel, "wselb": wsel.astype(ml_dtypes.bfloat16),
        "asc": asc, "wr": wr, "opw": opw, "fcw": fcw,
        "ident": np.eye(128, dtype=f32),
        "c1b": conv1_b.reshape(128, 1).copy(),
        "cdb": convd_b.reshape(2, 128).T.copy(),
        "dtb": dt_proj_b.reshape(2, 128).T.copy(),
        "dvec": Dv.reshape(2, 128).T.copy(),
        "fcb": fc_b.reshape(10, 1).copy(),
    }
    return consts


_CACHE = {}


def kernel(**inputs) -> np.ndarray:
    ids = np.asarray(inputs["ids"])
    assert ids.shape == (8, SEQ), ids.shape
    ids32 = np.ascontiguousarray(ids, dtype=np.int32)

    if "nc" not in _CACHE:
        _CACHE["nc"] = build_program()
    nc = _CACHE["nc"]

    consts = prep_consts(inputs)
    in_maps = []
    for b in range(8):
        m = dict(consts)
        m["ids"] = ids32[b].copy()
        in_maps.append(m)

    trace = os.environ.get("MAMBA_TRACE", "0") == "1"
    res = run_bass_kernel_spmd(nc, in_maps, core_ids=list(range(8)), trace=trace)
    _CACHE["last_results"] = res
    out = np.stack([res.results[b]["out"] for b in range(8)]).astype(np.float32)
    return out


# revision 14
# speedup vs baseline: 1.3068x; 1.1703x over previous
"""Trainium2 Bass kernel for CNN+Mamba classifier.

Contract: kernel(**inputs) takes FULL unsharded inputs (numpy), returns FULL
(8, 10) float32 output. Internally shards data-parallel over batch across 8
NeuronCores (1 example per core), with all parameters replicated.

Self-contained: hardcodes all shapes; no sibling imports.
"""

import os
from contextlib import ExitStack

import numpy as np
import ml_dtypes

import concourse.bass as bass
import concourse.bacc as bacc
import concourse.tile as tile
from concourse import mybir
from concourse.bass_utils import run_bass_kernel_spmd

FP = mybir.dt.float32
FR = mybir.dt.float32r
BF = mybir.dt.bfloat16
I32 = mybir.dt.int32

VOCAB, EMB, NCLS, SEQ = 50000, 256, 10, 2048
DM, DI, DS, DCONV, DTR = 128, 256, 16, 4, 8
L = SEQ // 2  # 1024 after maxpool
NTILE = DI // 8  # 32 scan tiles, each 8 channels x 16 states


def _rep_ap(t_ap, row0, nrows, rep, outer_rows=True):
    """AP reading `nrows` partition rows starting at row0 of a 2D SBUF tile,
    each repeated `rep` times. outer_rows=True -> dest p = row*rep + k;
    False -> dest p = k*nrows + row."""
    full = t_ap[:]
    pstep = full.ap[0][0]
    free = list(full.ap[1:])
    if outer_rows:
        dims = [[pstep, nrows], [0, rep]]
    else:
        dims = [[0, rep], [pstep, nrows]]
    return bass.AP(tensor=full.tensor, offset=full.offset + row0 * pstep,
                   ap=dims + free)


def _strided_pair(t_ap, n):
    """even/odd stride-2 APs over the free dim of a (128, 2n) tile."""
    full = t_ap[:]
    pstep = full.ap[0][0]
    ev = bass.AP(tensor=full.tensor, offset=full.offset, ap=[[pstep, 128], [2, n]])
    od = bass.AP(tensor=full.tensor, offset=full.offset + 1, ap=[[pstep, 128], [2, n]])
    return ev, od


def build_program():
    nc = bacc.Bacc("TRN2", target_bir_lowering=False, debug=False, num_devices=8)

    # ---- DRAM inputs (per-core) ----
    d_ids = nc.dram_tensor("ids", [SEQ], I32, kind="ExternalInput")
    d_emb = nc.dram_tensor("emb", [VOCAB, EMB], BF, kind="ExternalInput")
    d_c1w = nc.dram_tensor("c1w", [128, 5 * 2 * 128], BF, kind="ExternalInput")
    d_xcw = nc.dram_tensor("xcw", [128, 4 * 2 * 128], FR, kind="ExternalInput")
    d_zw = nc.dram_tensor("zw", [128, 2 * 128], FR, kind="ExternalInput")
    d_xpw = nc.dram_tensor("xpw", [128, 2 * 40], FR, kind="ExternalInput")
    d_dtw = nc.dram_tensor("dtw", [8, 2 * 128], FR, kind="ExternalInput")
    d_wsel = nc.dram_tensor("wsel", [128, 16 * 128], FR, kind="ExternalInput")
    d_wsel = nc.dram_tensor("wsel", [128, 16 * 128], FR, kind="ExternalInput")
    d_wselb = nc.dram_tensor("wselb", [128, 16 * 128], BF, kind="ExternalInput")
    d_asc = nc.dram_tensor("asc", [128, NTILE], FP, kind="ExternalInput")
    d_wr = nc.dram_tensor("wr", [128, 4 * 32], BF, kind="ExternalInput")
    d_opw = nc.dram_tensor("opw", [128, 2 * 128], FR, kind="ExternalInput")
    d_fcw = nc.dram_tensor("fcw", [128, NCLS], FP, kind="ExternalInput")
    d_ident = nc.dram_tensor("ident", [128, 128], BF, kind="ExternalInput")
    d_c1b = nc.dram_tensor("c1b", [128, 1], FP, kind="ExternalInput")
    d_cdb = nc.dram_tensor("cdb", [128, 2], FP, kind="ExternalInput")
    d_dtb = nc.dram_tensor("dtb", [128, 2], FP, kind="ExternalInput")
    d_dvec = nc.dram_tensor("dvec", [128, 2], FP, kind="ExternalInput")
    d_fcb = nc.dram_tensor("fcb", [10, 1], FP, kind="ExternalInput")

    d_out = nc.dram_tensor("out", [NCLS], FP, kind="ExternalOutput")

    Alu = mybir.AluOpType
    Act = mybir.ActivationFunctionType

    with ExitStack() as ctx:
        tc = ctx.enter_context(tile.TileContext(nc))
        W = ctx.enter_context(tc.tile_pool(name="w", bufs=1))

        # ---- load constants ----
        def load(dram, shape, dtype=FP):
            t = W.tile(list(shape), dtype, name=f"w_{dram.name}")
            nc.sync.dma_start(out=t[:], in_=dram[:])
            return t

        c1w = load(d_c1w, (128, 5 * 2 * 128), BF)
        xcw = load(d_xcw, (128, 4 * 2 * 128), FR)
        zw = load(d_zw, (128, 2 * 128), FR)
        xpw = load(d_xpw, (128, 2 * 40), FR)
        dtw = load(d_dtw, (8, 2 * 128), FR)
        wsel = load(d_wsel, (128, 16 * 128), FR)
        wsel = load(d_wsel, (128, 16 * 128), FR)
        wselb = load(d_wselb, (128, 16 * 128), BF)
        asc = load(d_asc, (128, NTILE))
        wr = load(d_wr, (128, 4 * 32), BF)
        opw = load(d_opw, (128, 2 * 128), FR)
        fcw = load(d_fcw, (128, NCLS))
        ident = load(d_ident, (128, 128), BF)
        c1b = load(d_c1b, (128, 1))
        cdb = load(d_cdb, (128, 2))
        dtb = load(d_dtb, (128, 2))
        dvec = load(d_dvec, (128, 2))
        fcb = load(d_fcb, (10, 1))

        # ids -> (128, 16): partition p holds ids[c*128+p] at column c
        ids_sb = W.tile([128, 16], I32)
        ids_src = bass.AP(tensor=d_ids[:].tensor, offset=0, ap=[[1, 128], [128, 16]])
        nc.sync.dma_start(out=ids_sb[:], in_=ids_src)

        # ---- persistent intermediates ----
        x_emb = [W.tile([128, SEQ + 4], BF, name=f"x_emb{_}") for _ in range(2)]  # pad 2 each side
        for h in range(2):
            nc.vector.memset(x_emb[h][:, 0:2], 0.0)
            nc.vector.memset(x_emb[h][:, SEQ + 2:SEQ + 4], 0.0)
        x_pool = W.tile([128, L + 3], FR)  # pad 3 left (causal dconv)
        nc.vector.memset(x_pool[:, 0:3].bitcast(FP), 0.0)
        relu_sb = W.tile([128, SEQ], BF)
        xs_sb = [W.tile([128, L], FR, name=f"xs_sb{_}") for _ in range(2)]
        sz_sb = [W.tile([128, L], FP, name=f"sz_sb{_}") for _ in range(2)]
        dt_sb = [W.tile([128, L], FR, name=f"dt_sb{_}") for _ in range(2)]
        u_sb = [W.tile([128, L], BF, name=f"u_sb{_}") for _ in range(2)]
        xdbl_sb = W.tile([40, L], FR)
        b_rep = W.tile([128, L], BF)
        c_rep = W.tile([128, L], BF)

        # ================= PHASE 1: embedding gather + transpose ============
        with tc.tile_pool(name="g", bufs=3) as gp, \
             tc.tile_pool(name="gt", bufs=4, space="PSUM") as gtp:
            for c in range(16):
                xg = gp.tile([128, EMB], BF)
                nc.gpsimd.indirect_dma_start(
                    out=xg[:], out_offset=None, in_=d_emb[:],
                    in_offset=bass.IndirectOffsetOnAxis(ap=ids_sb[:, c:c + 1], axis=0))
                for h in range(2):
                    pt = gtp.tile([128, 128], BF)
                    nc.tensor.transpose(out=pt[:], in_=xg[:, 128 * h:128 * (h + 1)],
                                        identity=ident[:])
                    nc.scalar.copy(out=x_emb[h][:, 2 + 128 * c:2 + 128 * (c + 1)],
                                   in_=pt[:])

        # ================= PHASE 2: conv1 + relu + maxpool ==================
        with tc.tile_pool(name="cp", bufs=1, space="PSUM") as cp:
            cps = cp.tile([128, SEQ], FP)  # 4 banks
            for nch in range(4):
                o = 512 * nch
                for k in range(5):
                    for kh in range(2):
                        nc.tensor.matmul(
                            out=cps[:, o:o + 512],
                            lhsT=c1w[:, (k * 2 + kh) * 128:(k * 2 + kh + 1) * 128],
                            rhs=x_emb[kh][:, o + k:o + k + 512],
                            start=(k == 0 and kh == 0), stop=(k == 4 and kh == 1))
            nc.scalar.activation(out=relu_sb[:], in_=cps[:], func=Act.Relu,
                                 bias=c1b[:, 0:1], scale=1.0)
        ev, od = _strided_pair(relu_sb, L)
        nc.vector.tensor_max(out=x_pool[:, 3:3 + L], in0=ev, in1=od)

        # ============ PHASE 3: in_proj (+ folded depthwise conv) + silu =====
        with tc.tile_pool(name="ip", bufs=1, space="PSUM") as ip:
            xcp = [ip.tile([128, L], FP, name=f"xcp{_}") for _ in range(2)]
            zp = [ip.tile([128, L], FP, name=f"zp{_}") for _ in range(2)]
            for h in range(2):
                for nch in range(2):
                    o = 512 * nch
                    for k in range(4):
                        nc.tensor.matmul(
                            out=xcp[h][:, o:o + 512],
                            lhsT=xcw[:, (k * 2 + h) * 128:(k * 2 + h + 1) * 128],
                            rhs=x_pool[:, o + k:o + k + 512],
                            start=(k == 0), stop=(k == 3))
                    nc.tensor.matmul(
                        out=zp[h][:, o:o + 512],
                        lhsT=zw[:, h * 128:(h + 1) * 128],
                        rhs=x_pool[:, 3 + o:3 + o + 512],
                        start=True, stop=True)
            for h in range(2):
                nc.scalar.activation(out=xs_sb[h][:], in_=xcp[h][:], func=Act.Silu,
                                     bias=cdb[:, h:h + 1], scale=1.0)
                nc.scalar.activation(out=sz_sb[h][:], in_=zp[h][:], func=Act.Silu,
                                     scale=1.0)

        # ================= PHASE 4: x_proj -> (dt_in, B, C) =================
        with tc.tile_pool(name="xp", bufs=1, space="PSUM") as xp:
            xdp = xp.tile([40, L], FP)
            for kh in range(2):
                for nch in range(2):
                    o = 512 * nch
                    nc.tensor.matmul(
                        out=xdp[:, o:o + 512],
                        lhsT=xpw[:, kh * 40:(kh + 1) * 40],
                        rhs=xs_sb[kh][:, o:o + 512],
                        start=(kh == 0), stop=(kh == 1))
            nc.vector.tensor_copy(out=xdbl_sb[:], in_=xdp[0:40, :])

        # ================= PHASE 5: dt (softplus) + u = dt*xs ===============
        with tc.tile_pool(name="dp", bufs=1, space="PSUM") as dp:
            dtp = [dp.tile([128, L], FP, name=f"dtp{_}") for _ in range(2)]
            for h in range(2):
                for nch in range(2):
                    o = 512 * nch
                    nc.tensor.matmul(
                        out=dtp[h][:, o:o + 512],
                        lhsT=dtw[0:8, h * 128:(h + 1) * 128],
                        rhs=xdbl_sb[0:8, o:o + 512],
                        start=True, stop=True)
            for h in range(2):
                # softplus(x+b) = ln(1 + exp(x+b)); x ~ -4 so no overflow
                nc.scalar.activation(out=dt_sb[h][:], in_=dtp[h][:], func=Act.Exp,
                                     bias=dtb[:, h:h + 1], scale=1.0)
                nc.scalar.activation(out=dt_sb[h][:], in_=dt_sb[h][:].bitcast(FP), func=Act.Ln,
                                     bias=1.0, scale=1.0)
        for h in range(2):
            nc.vector.tensor_mul(out=u_sb[h][:], in0=dt_sb[h][:].bitcast(FP), in1=xs_sb[h][:].bitcast(FP))

        # ================= PHASE 6: replicate B, C across channel groups ====
        # dest p = dl*16 + n  <-  bc row n (B: rows 0:16, C: rows 16:32)
        for dl in range(8):
            nc.gpsimd.dma_start(out=b_rep[dl * 16:(dl + 1) * 16, :],
                                in_=xdbl_sb[8:24, :].bitcast(FP))
            nc.gpsimd.dma_start(out=c_rep[dl * 16:(dl + 1) * 16, :],
                                in_=xdbl_sb[24:40, :].bitcast(FP))

        # ================= PHASE 7: selective scan ==========================
        with tc.tile_pool(name="yp", bufs=1, space="PSUM") as ypp, \
             tc.tile_pool(name="sg", bufs=1, space="PSUM") as sgp, \
             tc.tile_pool(name="sc", bufs=2) as scp:
            yp = [ypp.tile([128, L], FP, name=f"yp{_}") for _ in range(2)]  # 4 banks
            for i in range(NTILE):
                hh = i // 16
                lc = 8 * (i % 16)          # local channel base within half
                g = lc // 32               # 32-partition output group
                o = lc % 32                # offset inside group (0/8/16/24)
                v = o // 8                 # wr variant
                j = i % 16                 # selection variant
                dt_ps = sgp.tile([128, L], FP, tag="dt_ps")
                u_ps = sgp.tile([128, L], FP, tag="u_ps")
                for nch in range(2):
                    off = 512 * nch
                    nc.tensor.matmul(
                        out=dt_ps[:, off:off + 512],
                        lhsT=wsel[:, j * 128:(j + 1) * 128],
                        rhs=dt_sb[hh][:, off:off + 512],
                        start=True, stop=True)
                    nc.tensor.matmul(
                        out=u_ps[:, off:off + 512],
                        lhsT=wselb[:, j * 128:(j + 1) * 128],
                        rhs=u_sb[hh][:, off:off + 512],
                        start=True, stop=True)
                dA = scp.tile([128, L], BF, tag="dA")
                nc.scalar.activation(out=dA[:], in_=dt_ps[:], func=Act.Exp,
                                     scale=asc[:, i:i + 1])
                urep = scp.tile([128, L], BF, tag="urep")
                nc.scalar.copy(out=urep[:], in_=u_ps[:])
                dBu = scp.tile([128, L], BF, tag="dBu")
                nc.vector.tensor_mul(out=dBu[:], in0=urep[:], in1=b_rep[:])
                ht = scp.tile([128, L], BF, tag="ht")
                nc.vector.tensor_tensor_scan(out=ht[:], data0=dA[:], data1=dBu[:],
                                             initial=0.0, op0=Alu.mult, op1=Alu.add)
                hC = scp.tile([128, L], BF, tag="hC")
                nc.vector.tensor_mul(out=hC[:], in0=ht[:], in1=c_rep[:])
                for nch in range(2):
                    off = 512 * nch
                    nc.tensor.matmul(
                        out=yp[hh][32 * g:32 * (g + 1), off:off + 512],
                        lhsT=wr[:, v * 32:(v + 1) * 32],
                        rhs=hC[:, off:off + 512],
                        start=(o == 0), stop=(o == 24),
                        tile_position=(0, 32 * g))

            # ============= PHASE 8: gate, out_proj, mean, fc ================
            y2 = [W.tile([128, L], FR, name=f"y2{_}") for _ in range(2)]
            for h in range(2):
                y1 = scp.tile([128, L], FP, tag="y1")
                nc.vector.scalar_tensor_tensor(out=y1[:], in0=xs_sb[h][:].bitcast(FP),
                                               scalar=dvec[:, h:h + 1], in1=yp[h][:],
                                               op0=Alu.mult, op1=Alu.add)
                nc.vector.tensor_mul(out=y2[h][:], in0=y1[:], in1=sz_sb[h][:])

        with tc.tile_pool(name="op", bufs=1, space="PSUM") as opp:
            yop = opp.tile([128, L], FP)
            for h in range(2):
                for nch in range(2):
                    o = 512 * nch
                    nc.tensor.matmul(
                        out=yop[:, o:o + 512],
                        lhsT=opw[:, h * 128:(h + 1) * 128],
                        rhs=y2[h][:, o:o + 512],
                        start=(h == 0), stop=(h == 1))
            ymean = W.tile([128, 1], FP)
            nc.vector.tensor_reduce(out=ymean[:], in_=yop[:],
                                    axis=mybir.AxisListType.X, op=Alu.add)
            fcp = opp.tile([10, 1], FP)
            nc.tensor.matmul(out=fcp[:], lhsT=fcw[:, 0:NCLS], rhs=ymean[:],
                             start=True, stop=True)
            out_sb = W.tile([10, 1], FP)
            nc.vector.tensor_scalar_add(out=out_sb[:], in0=fcp[:],
                                        scalar1=fcb[0:10, 0:1])
        out_dst = bass.AP(tensor=d_out[:].tensor, offset=0, ap=[[1, NCLS]])
        out_src = bass.AP(tensor=out_sb[:].tensor, offset=out_sb[:].offset,
                          ap=[[out_sb[:].ap[0][0], NCLS]])
        nc.sync.dma_start(out=out_dst, in_=out_src)

    nc.compile()
    return nc


def prep_consts(inputs):
    """Host-side weight transforms (parameters only, no data-dependent work)."""
    f32 = np.float32
    emb = np.ascontiguousarray(np.asarray(inputs["emb"], f32).astype(ml_dtypes.bfloat16))
    conv1_w = np.asarray(inputs["conv1_w"], f32)      # (128, 256, 5)
    conv1_b = np.asarray(inputs["conv1_b"], f32)
    in_proj_w = np.asarray(inputs["in_proj_w"], f32)  # (512, 128)
    convd_w = np.asarray(inputs["convd_w"], f32)      # (256, 1, 4)
    convd_b = np.asarray(inputs["convd_b"], f32)
    x_proj_w = np.asarray(inputs["x_proj_w"], f32)    # (40, 256)
    dt_proj_w = np.asarray(inputs["dt_proj_w"], f32)  # (256, 8)
    dt_proj_b = np.asarray(inputs["dt_proj_b"], f32)
    A_log = np.asarray(inputs["A_log"], f32)          # (256, 16)
    Dv = np.asarray(inputs["D"], f32)
    out_proj_w = np.asarray(inputs["out_proj_w"], f32)  # (128, 256)
    fc_w = np.asarray(inputs["fc_w"], f32)            # (10, 128)
    fc_b = np.asarray(inputs["fc_b"], f32)

    c1w = np.zeros((128, 5, 2, 128), f32)
    for k in range(5):
        for kh in range(2):
            # lhsT[p, m] = conv1_w[m, kh*128+p, k]
            c1w[:, k, kh, :] = conv1_w[:, kh * 128:(kh + 1) * 128, k].T
    c1w = c1w.reshape(128, -1)

    Wx = in_proj_w[:DI]          # (256, 128)
    xcw = np.zeros((128, 4, 2, 128), f32)
    for k in range(4):
        Wxk = convd_w[:, 0, k][:, None] * Wx          # (256, 128)
        for mc in range(2):
            xcw[:, k, mc, :] = Wxk[mc * 128:(mc + 1) * 128, :].T
    xcw = xcw.reshape(128, -1)

    Wz = in_proj_w[DI:]
    zw = np.zeros((128, 2, 128), f32)
    for mc in range(2):
        zw[:, mc, :] = Wz[mc * 128:(mc + 1) * 128, :].T
    zw = zw.reshape(128, -1)

    xpw = np.zeros((128, 2, 40), f32)
    for kh in range(2):
        xpw[:, kh, :] = x_proj_w[:, kh * 128:(kh + 1) * 128].T
    xpw = xpw.reshape(128, -1)

    dtw = np.zeros((8, 2, 128), f32)
    for mc in range(2):
        dtw[:, mc, :] = dt_proj_w[mc * 128:(mc + 1) * 128, :].T
    dtw = dtw.reshape(8, -1)

    A = -np.exp(A_log)           # (256, 16)
    wsel = np.zeros((128, 16, 128), f32)
    for j in range(16):
        lc = 8 * j
        for dl in range(8):
            for n in range(DS):
                wsel[lc + dl, j, dl * 16 + n] = 1.0
    wsel = wsel.reshape(128, -1)
    asc = np.zeros((128, NTILE), f32)
    for i in range(NTILE):
        for p in range(128):
            asc[p, i] = A[8 * i + p // 16, p % 16]

    wr = np.zeros((128, 4, 32), f32)
    for v in range(4):
        for p in range(128):
            wr[p, v, 8 * v + p // 16] = 1.0
    wr = wr.reshape(128, -1).astype(ml_dtypes.bfloat16)

    opw = np.zeros((128, 2, 128), f32)
    for kh in range(2):
        opw[:, kh, :] = out_proj_w[:, kh * 128:(kh + 1) * 128].T
    opw = opw.reshape(128, -1)

    fcw = (fc_w / float(L)).T.copy()                  # (128, 10)

    consts = {
        "emb": emb,
        # noqa
        "c1w": c1w.astype(ml_dtypes.bfloat16), "xcw": xcw, "zw": zw, "xpw": xpw, "dtw": dtw,
        "wsel": wsel, "wselb": wsel.astype(ml_dtypes.bfloat16),
        "asc": asc, "wr": wr, "opw": opw, "fcw": fcw,
        "ident": np.eye(128, dtype=f32).astype(ml_dtypes.bfloat16),
        "c1b": conv1_b.reshape(128, 1).copy(),
        "cdb": convd_b.reshape(2, 128).T.copy(),
        "dtb": dt_proj_b.reshape(2, 128).T.copy(),
        "dvec": Dv.reshape(2, 128).T.copy(),
        "fcb": fc_b.reshape(10, 1).copy(),
    }
    return consts


_CACHE = {}


def kernel(**inputs) -> np.ndarray:
    ids = np.asarray(inputs["ids"])
    assert ids.shape == (8, SEQ), ids.shape
    ids32 = np.ascontiguousarray(ids, dtype=np.int32)

    if "nc" not in _CACHE:
        _CACHE["nc"] = build_program()
    nc = _CACHE["nc"]

    consts = prep_consts(inputs)
    in_maps = []
    for b in range(8):
        m = dict(consts)
        m["ids"] = ids32[b].copy()
        in_maps.append(m)

    trace = os.environ.get("MAMBA_TRACE", "0") == "1"
    res = run_bass_kernel_spmd(nc, in_maps, core_ids=list(range(8)), trace=trace)
    _CACHE["last_results"] = res
    out = np.stack([res.results[b]["out"] for b in range(8)]).astype(np.float32)
    return out
